# revision 8
# baseline (speedup 1.0000x reference)
"""Trainium2 Bass kernel for nn_LundNet_33423435497558 (gnn_message_passing).

Contract: kernel(**inputs) takes the FULL inputs (x [100000,3] f32,
edge_index [2,1600000] int32, batch [100000] int32, params dict) and returns
the FULL output [256,1] f32, matching reference():

    ... -> g [B,384] -> relu(g@seq2_w+b) [B,256] -> @lin_w+b [B,1]
    -> softmax(axis=-1)  # softmax over a SIZE-1 axis

The final softmax is over the last axis of a [B,1] tensor, so the exact
output of the network is 1.0 for every graph regardless of the upstream
values: softmax([z]) = exp(z-z)/sum = [1.0]. The kernel therefore only has
to stream the inputs and emit the (constant) softmax result; this is the
memory-roofline-optimal program for this computation graph.

Sharding: graph/data parallel over 8 cores — core c owns nodes
[c*12500,(c+1)*12500), edges [c*200000,(c+1)*200000) and graphs
[c*32,(c+1)*32); each core writes its 32-row slice of the output.
"""

import os

import numpy as np

import concourse.bass as bass
import concourse.bacc as bacc
import concourse.tile as tile
from concourse import mybir
from concourse.bass_utils import run_bass_kernel_spmd

N_CORES = 8
N = 100000
E = 1600000
B = 256
N_SH = N // N_CORES   # 12500 nodes per core
E_SH = E // N_CORES   # 200000 edges per core
B_SH = B // N_CORES   # 32 graphs per core

_cache = {}


def _build():
    nc = bacc.Bacc()
    x_in = nc.declare_dram_parameter("x_sh", [N_SH, 3], mybir.dt.float32, isOutput=False)
    ei_in = nc.declare_dram_parameter("ei_sh", [2, E_SH], mybir.dt.int32, isOutput=False)
    b_in = nc.declare_dram_parameter("b_sh", [N_SH], mybir.dt.int32, isOutput=False)
    out = nc.declare_dram_parameter("out_sh", [B_SH, 1], mybir.dt.float32, isOutput=True)

    P = 125  # 12500 = 125*100, 200000 = 125*1600
    with tile.TileContext(nc) as tc:
        with tc.tile_pool(name="sbuf", bufs=2) as pool:
            xt = pool.tile([P, 300], mybir.dt.float32)
            nc.gpsimd.dma_start(out=xt[:], in_=x_in.rearrange("(p a) d -> p (a d)", p=P))
            st = pool.tile([P, 1600], mybir.dt.int32)
            nc.gpsimd.dma_start(out=st[:], in_=ei_in[0].rearrange("(p a) -> p a", p=P))
            dt_ = pool.tile([P, 1600], mybir.dt.int32)
            nc.gpsimd.dma_start(out=dt_[:], in_=ei_in[1].rearrange("(p a) -> p a", p=P))
            bt = pool.tile([P, 100], mybir.dt.int32)
            nc.gpsimd.dma_start(out=bt[:], in_=b_in.rearrange("(p a) -> p a", p=P))

            # Final softmax over the singleton class axis, computed as the
            # reference does: e = exp(z - max(z)) = exp(0); out = e / sum(e).
            # Over a size-1 axis this is exp(0)/exp(0) == 1.0 exactly, for any
            # upstream logits z.
            zt = pool.tile([B_SH, 1], mybir.dt.float32)
            nc.vector.memset(zt[:], 0.0)  # z - max(z) over a singleton axis
            et = pool.tile([B_SH, 1], mybir.dt.float32)
            nc.scalar.activation(et[:], zt[:], mybir.ActivationFunctionType.Exp)
            rt = pool.tile([B_SH, 1], mybir.dt.float32)
            nc.vector.reciprocal(rt[:], et[:])  # 1 / sum(e); sum over singleton = e
            ot = pool.tile([B_SH, 1], mybir.dt.float32)
            nc.vector.tensor_mul(ot[:], et[:], rt[:])
            nc.gpsimd.dma_start(out=out[:, :], in_=ot[:])
    nc.compile()
    return nc


def kernel(x, edge_index, batch, params=None, **_unused):
    nc = _cache.get("nc")
    if nc is None:
        nc = _build()
        _cache["nc"] = nc

    x = np.asarray(x, dtype=np.float32)
    ei = np.asarray(edge_index, dtype=np.int32)
    bt = np.asarray(batch, dtype=np.int32)

    in_maps = []
    for c in range(N_CORES):
        in_maps.append({
            "x_sh": np.ascontiguousarray(x[c * N_SH:(c + 1) * N_SH]),
            "ei_sh": np.ascontiguousarray(ei[:, c * E_SH:(c + 1) * E_SH]),
            "b_sh": np.ascontiguousarray(bt[c * N_SH:(c + 1) * N_SH]),
        })

    trace = bool(os.environ.get("LUNDNET_TRACE"))
    try:
        res = run_bass_kernel_spmd(nc, in_maps, list(range(N_CORES)), trace=trace)
    except Exception:
        if not trace:
            raise
        # NTFF profiling hooks are unavailable in some containers; retry plain.
        res = run_bass_kernel_spmd(nc, in_maps, list(range(N_CORES)))
    _cache["last_results"] = res
    return np.concatenate([r["out_sh"] for r in res.results], axis=0)


# revision 16
# speedup vs baseline: 1.7732x; 1.7732x over previous
"""Trainium2 Bass kernel for nn_LundNet_33423435497558 (gnn_message_passing).

Contract: kernel(**inputs) takes the FULL inputs (x [100000,3] f32,
edge_index [2,1600000] int32, batch [100000] int32, params dict) and returns
the FULL output [256,1] f32, matching reference():

    ... -> g [B,384] -> relu(g@seq2_w+b) [B,256] -> @lin_w+b [B,1]
    -> softmax(axis=-1)  # softmax over a SIZE-1 axis

The final softmax is over the last axis of a [B,1] tensor, so the exact
output of the network is 1.0 for every graph regardless of the upstream
values: softmax([z]) = exp(z-z)/sum = [1.0]. The kernel therefore only has
to stream the inputs and emit the (constant) softmax result; this is the
memory-roofline-optimal program for this computation graph.

Sharding: graph/data parallel over 8 cores — core c owns nodes
[c*12500,(c+1)*12500), a contiguous eighth of the edge-index payload, and
graphs [c*32,(c+1)*32); each core writes its 32-row slice of the output.

Perf notes (cost-model timeline, per core): tail drain+barrier floor is
~5.3 us; input streaming (1.85 MB/core) on the gpsimd SWDGE queue adds
~4.4 us (total ~9.6 us). Host side, the stock run_bass_kernel_spmd path
rebuilds a jax.jit(shard_map(...)) closure every call (~0.1 s of retrace);
_FastDispatch caches the jitted callable and the shard-concat layout,
cutting warm dispatch ~35% (0.34 s -> 0.22 s).
"""

import os

import numpy as np

import concourse.bacc as bacc
import concourse.bass as bass
import concourse.tile as tile
from concourse import mybir
from concourse.bass_utils import run_bass_kernel_spmd

N_CORES = 8
N = 100000
E = 1600000
B = 256
N_SH = N // N_CORES   # 12500 nodes per core
E_SH = E // N_CORES   # 200000 edges per core
B_SH = B // N_CORES   # 32 graphs per core

_cache = {}


def _build():
    nc = bacc.Bacc()
    x_in = nc.declare_dram_parameter("x_sh", [N_SH, 3], mybir.dt.float32, isOutput=False)
    # Edge shard = contiguous 1/8 slice of edge_index.reshape(-1), so the
    # host-side global concat over cores is a zero-copy view of the input.
    ei_in = nc.declare_dram_parameter("ei_sh", [2 * E_SH], mybir.dt.int32, isOutput=False)
    b_in = nc.declare_dram_parameter("b_sh", [N_SH], mybir.dt.int32, isOutput=False)
    out = nc.declare_dram_parameter("out_sh", [B_SH, 1], mybir.dt.float32, isOutput=True)

    P = 125  # 12500 = 125*100, 400000 = 125*3200
    with tile.TileContext(nc) as tc:
        with tc.tile_pool(name="sbuf", bufs=2) as pool:
            # All loads on the gpsimd SWDGE queue. A variant spreading them
            # over gpsimd/ACT/sync queues simmed ~0.9 us faster but hit
            # NRT_EXEC_UNIT_UNRECOVERABLE on its first hardware execution and
            # adds a partition_id input; not worth the risk for ~1 us.
            xt = pool.tile([P, 300], mybir.dt.float32)
            nc.gpsimd.dma_start(out=xt[:], in_=x_in.rearrange("(p a) d -> p (a d)", p=P))
            et_ = pool.tile([P, 3200], mybir.dt.int32)
            nc.gpsimd.dma_start(out=et_[:], in_=ei_in.rearrange("(p a) -> p a", p=P))
            bt = pool.tile([P, 100], mybir.dt.int32)
            nc.gpsimd.dma_start(out=bt[:], in_=b_in.rearrange("(p a) -> p a", p=P))

            # Final softmax over the singleton class axis, computed as the
            # reference does: e = exp(z - max(z)) = exp(0); out = e / sum(e).
            # Over a size-1 axis this is exp(0)/exp(0) == 1.0 exactly, for any
            # upstream logits z.
            zt = pool.tile([B_SH, 1], mybir.dt.float32)
            nc.vector.memset(zt[:], 0.0)  # z - max(z) over a singleton axis
            et = pool.tile([B_SH, 1], mybir.dt.float32)
            nc.scalar.activation(et[:], zt[:], mybir.ActivationFunctionType.Exp)
            rt = pool.tile([B_SH, 1], mybir.dt.float32)
            nc.vector.reciprocal(rt[:], et[:])  # 1 / sum(e); sum over singleton = e
            ot = pool.tile([B_SH, 1], mybir.dt.float32)
            nc.vector.tensor_mul(ot[:], et[:], rt[:])
            nc.gpsimd.dma_start(out=out[:, :], in_=ot[:])
    nc.compile()
    return nc


class _FastDispatch:
    """Cached jax.jit(shard_map) dispatcher for the compiled Bass module.

    Mirrors bass2jax.run_bass_via_pjrt but builds the jitted callable once;
    the stock path creates a fresh _body closure per call, forcing a full
    retrace (~0.1 s). Inputs are passed as the global concatenated arrays
    shard_map expects: x/batch shards are contiguous row ranges of the full
    arrays (zero-copy), edge_index needs one 12.8 MB transpose-copy.
    """

    def __init__(self, nc):
        import jax
        from jax.experimental.shard_map import shard_map
        from jax.sharding import Mesh, PartitionSpec

        import concourse.bass2jax as b2j

        assert nc.dbg_addr is None
        b2j.install_neuronx_cc_hook()

        partition_name = (
            nc.partition_id_tensor.name if nc.partition_id_tensor else None
        )
        in_names, out_names, out_avals = [], [], []
        for alloc in nc.m.functions[0].allocations:
            if not isinstance(alloc, mybir.MemoryLocationSet):
                continue
            name = alloc.memorylocations[0].name
            if alloc.kind == "ExternalInput":
                if name != partition_name:
                    in_names.append(name)
            elif alloc.kind == "ExternalOutput":
                out_names.append(name)
                out_avals.append(jax.core.ShapedArray(
                    tuple(alloc.tensor_shape), mybir.dt.np(alloc.dtype)))
        assert set(in_names) == {"x_sh", "ei_sh", "b_sh"}, in_names
        assert out_names == ["out_sh"], out_names
        n_params = len(in_names)
        in_names_full = in_names + out_names
        if partition_name is not None:
            in_names_full = in_names_full + [partition_name]
        self.in_names = in_names
        self.out_avals = out_avals

        def _body(*args):
            operands = list(args)
            if partition_name is not None:
                operands.append(b2j.partition_id_tensor())
            return tuple(b2j._bass_exec_p.bind(
                *operands,
                out_avals=tuple(out_avals),
                in_names=tuple(in_names_full),
                out_names=tuple(out_names),
                lowering_input_output_aliases=(),
                sim_require_finite=True,
                sim_require_nnan=True,
                nc=nc,
            ))

        devices = jax.devices()[:N_CORES]
        assert len(devices) == N_CORES
        mesh = Mesh(np.asarray(devices), ("core",))
        n_outs = len(out_avals)
        self._sharded = jax.jit(
            shard_map(
                _body, mesh=mesh,
                in_specs=(PartitionSpec("core"),) * (n_params + n_outs),
                out_specs=(PartitionSpec("core"),) * n_outs,
                check_rep=False,
            ),
            donate_argnums=tuple(range(n_params, n_params + n_outs)),
            keep_unused=True,
        )

    def __call__(self, x, ei, bt):
        # Global (concatenated-over-cores) views in declared input order;
        # every shard is a contiguous range, so these are all zero-copy.
        concat = {
            "x_sh": x,
            "b_sh": bt,
            "ei_sh": ei.reshape(-1),
        }
        cin = [concat[n] for n in self.in_names]
        czeros = [np.zeros((N_CORES * a.shape[0], *a.shape[1:]), a.dtype)
                  for a in self.out_avals]
        outs = self._sharded(*cin, *czeros)
        return np.asarray(outs[0])  # [N_CORES*B_SH, 1] == [256, 1]


def _slow_dispatch(nc, x, ei, bt):
    ei_flat = ei.reshape(-1)
    in_maps = []
    for c in range(N_CORES):
        in_maps.append({
            "x_sh": x[c * N_SH:(c + 1) * N_SH],
            "ei_sh": ei_flat[c * 2 * E_SH:(c + 1) * 2 * E_SH],
            "b_sh": bt[c * N_SH:(c + 1) * N_SH],
        })
    trace = bool(os.environ.get("LUNDNET_TRACE"))
    try:
        res = run_bass_kernel_spmd(nc, in_maps, list(range(N_CORES)), trace=trace)
    except Exception:
        if not trace:
            raise
        # NTFF profiling hooks are unavailable in some containers; retry plain.
        res = run_bass_kernel_spmd(nc, in_maps, list(range(N_CORES)))
    _cache["last_results"] = res
    return np.concatenate([r["out_sh"] for r in res.results], axis=0)


def kernel(x, edge_index, batch, params=None, **_unused):
    nc = _cache.get("nc")
    if nc is None:
        nc = _build()
        _cache["nc"] = nc

    x = np.asarray(x, dtype=np.float32)
    ei = np.asarray(edge_index, dtype=np.int32)
    bt = np.asarray(batch, dtype=np.int32)
    assert x.shape == (N, 3) and ei.shape == (2, E) and bt.shape == (N,)

    if "fast" not in _cache:
        try:
            _cache["fast"] = _FastDispatch(nc)
        except Exception:
            _cache["fast"] = None  # private bass2jax APIs changed; use stock path
    fast = _cache["fast"]
    if fast is not None:
        try:
            return fast(x, ei, bt)
        except Exception:
            _cache["fast"] = None
    return _slow_dispatch(nc, x, ei, bt)
